# revision 5
# baseline (speedup 1.0000x reference)
"""Trainium2 Bass kernel for nn_NeRFLoss (data-parallel over 8 NeuronCores).

Sharding: pure data parallel along n_rays. Each core handles 8192 rays.
Scalar loss terms (distortion / CE) are computed as per-partition partial
sums on device and combined on host (float64).

Distortion math (per ray, S=192 samples, z sorted):
    m4  := 4*m  = z_prev + 2 z + z_next   (edges clamped)
    iv2 := 2*dt = z_next - z_prev         (edges clamped)
    4*loss = W*WM4 + sum(w^2 * m4) + (2/3) sum(w^2 * iv2) - 2 sum(w * cwm4)
where W = sum(w), wm4 = w*m4, cwm4 = inclusive cumsum(wm4), WM4 = sum(wm4).
sum(w^2 * iv2) is computed as sum(z * d) with d the adjoint stencil of iv2
applied to sq = w^2.  The cumsum is chained across the 8 ray-groups that
share a partition row; boundary values B_g of the chained scan give both
WM4_g = B_g - B_{g-1} and the chain correction  sum_g B_{g-1} * W_g.
"""

import numpy as np

import concourse.bass as bass
import concourse.mybir as mybir
from concourse.tile import TileContext
from concourse.bass_utils import run_bass_kernel_spmd

N_RAYS, S, C = 65536, 192, 7
N_CORES = 8
N_LOC = N_RAYS // N_CORES        # 8192 rays per core
T_TILES = 8                      # distortion mega-tiles per core
G = 8                            # ray-groups of 128 per mega-tile
FD = G * S                       # 1536 free elements per partition
GC = N_LOC // 128                # 64 rays per partition (grouped layout)
LAMBDA_DIST = 1e-4

F32 = mybir.dt.float32
Alu = mybir.AluOpType
Act = mybir.ActivationFunctionType
AX = mybir.AxisListType

# parts[128, P_COLS] column layout (per-partition partial sums)
C_S1, C_S2, C_S3, C_WB, C_CR = 0, 8, 16, 24, 32
C_LSE, C_PICK = 40, 41
P_COLS = 42

_CACHE = {}


def _legalize_waits(nc, max_waits=1):
    """This walrus build accepts a single embedded sync-wait per TPB
    instruction ("Too many sync wait commands").  Hoist excess waits onto
    one-wait NoOps inserted just before the instruction in its engine
    stream — same semantics, sems are >= monotonic counters."""
    k = 0
    for fn in nc.m.functions:
        for bb in fn.blocks:
            out = []
            for ins in bb.instructions:
                si = getattr(ins, "sync_info", None)
                waits = list(si.on_wait) if si is not None and si.on_wait else []
                if len(waits) > max_waits:
                    for w in waits[:-max_waits]:
                        k += 1
                        out.append(mybir.InstNoOp(
                            name=f"waitsplit-{k}",
                            ins=[], outs=[],
                            engine=ins.engine,
                            sync_info=mybir.SyncInfo(on_wait=[w], on_update=[]),
                        ))
                    ins.sync_info = mybir.SyncInfo(
                        on_wait=waits[-max_waits:], on_update=si.on_update)
                out.append(ins)
            if len(out) != len(bb.instructions):
                bb.instructions[:] = out
    return k


def build_nc(legalize=True):
    nc = bass.Bass("TRN2", target_bir_lowering=False)

    w_d = nc.dram_tensor("w", [N_LOC, S], F32, kind="ExternalInput")
    z_d = nc.dram_tensor("z", [N_LOC, S], F32, kind="ExternalInput")
    sem_d = nc.dram_tensor("sem", [128, GC * C], F32, kind="ExternalInput")
    oh_d = nc.dram_tensor("onehot", [128, GC * C], F32, kind="ExternalInput")
    dep_d = nc.dram_tensor("dep", [128, GC], F32, kind="ExternalInput")
    mk4_d = nc.dram_tensor("mask4", [128, GC], F32, kind="ExternalInput")
    rgb_d = nc.dram_tensor("rgb", [128, 192], F32, kind="ExternalInput")
    tgt_d = nc.dram_tensor("tgt", [128, 192], F32, kind="ExternalInput")

    rgbo_d = nc.dram_tensor("rgb_out", [128, 192], F32, kind="ExternalOutput")
    sky_d = nc.dram_tensor("sky_out", [128, GC], F32, kind="ExternalOutput")
    parts_d = nc.dram_tensor("parts", [128, P_COLS], F32, kind="ExternalOutput")

    v = nc.vector
    sc = nc.scalar

    with TileContext(nc) as tc:
        with tc.tile_pool(name="io", bufs=2) as io, \
             tc.tile_pool(name="work", bufs=2) as work, \
             tc.tile_pool(name="small", bufs=2) as small, \
             tc.tile_pool(name="acc", bufs=1) as accp:

            parts = accp.tile([128, P_COLS], F32)

            # ---------------- small per-ray losses ----------------
            # rgb: (rgb - tgt)^2
            rgbt = small.tile([128, 192], F32, tag="rgbt")
            tgtt = small.tile([128, 192], F32, tag="tgtt")
            nc.sync.dma_start(rgbt[:], rgb_d[:])
            nc.sync.dma_start(tgtt[:], tgt_d[:])
            df = small.tile([128, 192], F32, tag="df")
            v.tensor_sub(df[:], rgbt[:], tgtt[:])
            sc.activation(df[:], df[:], Act.Square)
            nc.sync.dma_start(rgbo_d[:], df[:])

            # sky: 0.1 * mask4 * exp(-depth)
            dept = small.tile([128, GC], F32, tag="dept")
            mk4t = small.tile([128, GC], F32, tag="mk4t")
            nc.sync.dma_start(dept[:], dep_d[:])
            nc.sync.dma_start(mk4t[:], mk4_d[:])
            e4 = small.tile([128, GC], F32, tag="e4")
            sc.activation(e4[:], dept[:], Act.Exp, scale=-1.0)
            skyt = small.tile([128, GC], F32, tag="skyt")
            v.scalar_tensor_tensor(skyt[:], mk4t[:], 0.1, e4[:], Alu.mult, Alu.mult)
            nc.sync.dma_start(sky_d[:], skyt[:])

            # CE partials: sum(lse) and sum(onehot*sem)
            semt = small.tile([128, GC * C], F32, tag="semt")
            oht = small.tile([128, GC * C], F32, tag="oht")
            nc.sync.dma_start(semt[:], sem_d[:])
            nc.sync.dma_start(oht[:], oh_d[:])
            expt = small.tile([128, GC * C], F32, tag="expt")
            sc.activation(expt[:], semt[:], Act.Exp)
            ssum = small.tile([128, GC], F32, tag="ssum")
            v.reduce_sum(ssum[:], expt[:].rearrange("p (g c) -> p g c", c=C),
                         axis=AX.X)
            lses = small.tile([128, GC], F32, tag="lses")
            sc.activation(lses[:], ssum[:], Act.Ln,
                          accum_out=parts[:, C_LSE:C_LSE + 1])
            pick = small.tile([128, GC * C], F32, tag="pick")
            v.scalar_tensor_tensor(pick[:], oht[:], 1.0, semt[:],
                                   Alu.mult, Alu.mult,
                                   accum_out=parts[:, C_PICK:C_PICK + 1])

            # ---------------- distortion ----------------
            for t in range(T_TILES):
                w = io.tile([128, FD], F32, tag="w")
                z = io.tile([128, FD], F32, tag="z")
                w3 = w[:].rearrange("p (g s) -> p g s", g=G)
                z3 = z[:].rearrange("p (g s) -> p g s", g=G)
                src_w = w_d[1024 * t:1024 * (t + 1), :].rearrange(
                    "(g p) s -> p g s", p=128)
                src_z = z_d[1024 * t:1024 * (t + 1), :].rearrange(
                    "(g p) s -> p g s", p=128)
                nc.sync.dma_start(w3, src_w)
                nc.sync.dma_start(z3, src_z)

                # u1 = 2z + z_prev (col0: 3*z0)
                u1 = work.tile([128, FD], F32, tag="u1")
                u13 = u1[:].rearrange("p (g s) -> p g s", g=G)
                v.scalar_tensor_tensor(u13[:, :, 1:S], z3[:, :, 1:S], 2.0,
                                       z3[:, :, 0:S - 1], Alu.mult, Alu.add)
                v.tensor_scalar_mul(u13[:, :, 0:1], z3[:, :, 0:1], 3.0)

                # m4 = u1 + z_next (col S-1: u1 + z_{S-1})
                m4 = work.tile([128, FD], F32, tag="m4")
                m43 = m4[:].rearrange("p (g s) -> p g s", g=G)
                v.tensor_add(m43[:, :, 0:S - 1], z3[:, :, 1:S],
                             u13[:, :, 0:S - 1])
                v.tensor_add(m43[:, :, S - 1:S], z3[:, :, S - 1:S],
                             u13[:, :, S - 1:S])

                # wm4 = w * m4 ; chained inclusive cumsum
                wm4 = work.tile([128, FD], F32, tag="wm4")
                v.tensor_mul(wm4[:], w[:], m4[:])
                cwm4 = work.tile([128, FD], F32, tag="cwm4")
                v.tensor_tensor_scan(cwm4[:], wm4[:], wm4[:], 0.0,
                                     Alu.add, Alu.bypass)
                cwm43 = cwm4[:].rearrange("p (g s) -> p g s", g=G)

                # per-group W
                W8 = small.tile([128, G], F32, tag="W8")
                v.reduce_sum(W8[:], w3, axis=AX.X)

                # sq = w^2 (scalar engine)
                sq = work.tile([128, FD], F32, tag="sq")
                sc.activation(sq[:], w[:], Act.Square)

                # d = adjoint stencil of iv2 applied to sq
                d = work.tile([128, FD], F32, tag="d")
                d3 = d[:].rearrange("p (g s) -> p g s", g=G)
                sq3 = sq[:].rearrange("p (g s) -> p g s", g=G)
                v.tensor_sub(d3[:, :, 1:S - 1], sq3[:, :, 0:S - 2],
                             sq3[:, :, 2:S])
                v.scalar_tensor_tensor(d3[:, :, 0:1], sq3[:, :, 1:2], -1.0,
                                       sq3[:, :, 0:1], Alu.mult, Alu.subtract)
                v.tensor_add(d3[:, :, S - 1:S], sq3[:, :, S - 2:S - 1],
                             sq3[:, :, S - 1:S])

                # fused product+row-sum partials
                scr = work.tile([128, FD], F32, tag="scr")
                v.scalar_tensor_tensor(scr[:], sq[:], 1.0, m4[:],
                                       Alu.mult, Alu.mult,
                                       accum_out=parts[:, C_S1 + t:C_S1 + t + 1])
                scr2 = work.tile([128, FD], F32, tag="scr2")
                v.scalar_tensor_tensor(scr2[:], d[:], 1.0, z[:],
                                       Alu.mult, Alu.mult,
                                       accum_out=parts[:, C_S2 + t:C_S2 + t + 1])
                scr3 = work.tile([128, FD], F32, tag="scr3")
                v.scalar_tensor_tensor(scr3[:], w[:], 1.0, cwm4[:],
                                       Alu.mult, Alu.mult,
                                       accum_out=parts[:, C_S3 + t:C_S3 + t + 1])

                # scan boundary values B_g ; WB and chain-correction terms
                B = small.tile([128, G], F32, tag="B")
                sc.copy(B[:].rearrange("p (g o) -> p g o", o=1),
                        cwm43[:, :, S - 1:S])
                wbs = small.tile([128, G], F32, tag="wbs")
                v.scalar_tensor_tensor(wbs[:], W8[:], 1.0, B[:],
                                       Alu.mult, Alu.mult,
                                       accum_out=parts[:, C_WB + t:C_WB + t + 1])
                crs = small.tile([128, G - 1], F32, tag="crs")
                v.scalar_tensor_tensor(crs[:], W8[:, 1:G], 1.0, B[:, 0:G - 1],
                                       Alu.mult, Alu.mult,
                                       accum_out=parts[:, C_CR + t:C_CR + t + 1])

            nc.sync.dma_start(parts_d[:], parts[:])

    if legalize:
        _legalize_waits(nc)
    nc.finalize()
    return nc


def _pack_core(core, rgb0, ws0, z_vals0, semantic0, depth0, target_rgb,
               onehot, mask4):
    lo, hi = core * N_LOC, (core + 1) * N_LOC
    return {
        "w": np.ascontiguousarray(ws0[lo:hi]),
        "z": np.ascontiguousarray(z_vals0[lo:hi]),
        "sem": np.ascontiguousarray(semantic0[lo:hi]).reshape(128, GC * C),
        "onehot": np.ascontiguousarray(onehot[lo:hi]).reshape(128, GC * C),
        "dep": np.ascontiguousarray(depth0[lo:hi]).reshape(128, GC),
        "mask4": np.ascontiguousarray(mask4[lo:hi]).reshape(128, GC),
        "rgb": np.ascontiguousarray(rgb0[lo:hi]).reshape(128, 192),
        "tgt": np.ascontiguousarray(target_rgb[lo:hi]).reshape(128, 192),
    }


def assemble(results):
    """Combine per-core outputs into the reference's return structure."""
    rgb_loss = np.concatenate(
        [r["rgb_out"].reshape(N_LOC, 3) for r in results], axis=0)
    sky = np.concatenate(
        [r["sky_out"].reshape(N_LOC) for r in results], axis=0)

    dist_sum = 0.0
    lse_sum = 0.0
    pick_sum = 0.0
    for r in results:
        p = r["parts"].astype(np.float64)
        s1 = p[:, C_S1:C_S1 + 8].sum()
        s2 = p[:, C_S2:C_S2 + 8].sum()
        s3 = p[:, C_S3:C_S3 + 8].sum()
        wb = p[:, C_WB:C_WB + 8].sum()
        cr = p[:, C_CR:C_CR + 8].sum()
        dist_sum += 0.25 * (wb + cr + s1 + (2.0 / 3.0) * s2 - 2.0 * s3)
        lse_sum += p[:, C_LSE].sum()
        pick_sum += p[:, C_PICK].sum()

    dist = np.float32(LAMBDA_DIST * dist_sum / N_RAYS)
    ce = np.float32(0.04 * (lse_sum - pick_sum) / N_RAYS)
    return rgb_loss, dist, ce, sky


def host_inputs(rgb0, opacity0, ws0, z_vals0, semantic0, depth0, target_rgb,
                label, stages):
    """Host-side marshalling: cast/shard the full inputs into per-core maps."""
    rgb0 = np.asarray(rgb0, dtype=np.float32)
    ws0 = np.asarray(ws0, dtype=np.float32)
    z_vals0 = np.asarray(z_vals0, dtype=np.float32)
    semantic0 = np.asarray(semantic0, dtype=np.float32)
    depth0 = np.asarray(depth0, dtype=np.float32)
    target_rgb = np.asarray(target_rgb, dtype=np.float32)
    label = np.asarray(label)
    onehot = (label[:, None] == np.arange(C)[None, :]).astype(np.float32)
    mask4 = (label == 4).astype(np.float32)
    return [
        _pack_core(c, rgb0, ws0, z_vals0, semantic0, depth0, target_rgb,
                   onehot, mask4)
        for c in range(N_CORES)
    ]


def kernel(rgb0, opacity0, ws0, z_vals0, semantic0, depth0, target_rgb,
           label, stages, _trace=False):
    if "nc" not in _CACHE:
        _CACHE["nc"] = build_nc()
    nc = _CACHE["nc"]
    in_maps = host_inputs(rgb0, opacity0, ws0, z_vals0, semantic0, depth0,
                          target_rgb, label, stages)
    res = run_bass_kernel_spmd(nc, in_maps, core_ids=list(range(N_CORES)),
                               trace=_trace)
    out = assemble(res.results)
    if _trace:
        return out, res
    return out
